# revision 1
# baseline (speedup 1.0000x reference)
"""Supervised contrastive loss on 8 Trainium2 NeuronCores.

Strategy (data-parallel over embedding rows, per the sharding hint), with a
label-sorted layout so the masked work collapses to narrow windows:

  - The host sorts rows AND columns by label (the loss is permutation
    invariant). Each core owns 512 sorted rows; each 128-row m-tile's
    same-label partners then live in ONE contiguous column window of at
    most 512 columns (multinomial counts make wider windows essentially
    impossible; asserted on the host).
  - Dense path: bf16 PE matmuls compute the [128, B] similarity slab in
    PSUM chunks; one ACT Exp pass per chunk (scale=1/T) with the fused
    per-row accumulate yields sum_j exp(s_ij). The Exp output is a dead
    store - only the accumulator is consumed.
  - Window path: 4 small matmuls recompute the window's sims (bit-identical
    inputs), then exp / is_equal mask / masked-multiply / row-reduce /
    log1p run on [128, 512] tiles only (~1/8 of the columns):
      sum_same = sum_win mask * exp;  denom = sum_all - sum_same
      slog = sum_win ln(1 + me * (1/denom))   [diagonal included]
  - Per-row loss: cnt_i*ln(denom_i) + slog_i - ln(denom_i + e^{s_ii})
                  - sum_{j same} s_ij + s_ii
    where cnt, s_ii, e^{s_ii} and sum_{j same} s_ij (via class-sum matrix
    G) are exact O(B*D) host precomputes.
  - Each core writes its 512 per-row contributions; the host sums 4096
    values and divides by num_pos (exact, from label counts).
"""

import ml_dtypes
import numpy as np

import concourse.bass as bass
import concourse.bacc as bacc
import concourse.mybir as mybir
import concourse.tile as tile
from concourse.bass_utils import run_bass_kernel_spmd

B = 4096          # total rows
D = 512           # embedding dim
NCORES = 8
BL = B // NCORES  # rows per core
NK = D // 128     # contraction k-tiles
NMT = BL // 128   # output m-tiles per core
CH = 1024         # dense column chunk (2 PSUM banks)
NCH = B // CH     # dense chunks per m-tile row
WIN = 512         # same-label column window per m-tile
TINV = 10.0       # 1 / temperature
F32 = mybir.dt.float32
BF16 = mybir.dt.bfloat16
F8 = mybir.dt.float8e4
NP_F8 = mybir.dt.np(F8)
SCALE = 16.0      # fp8 pre-scale; folded out via the Exp activation scale

_CACHE = {}


def _build_nc():
    nc = bacc.Bacc()
    NKK = NK // 2     # DoubleRow k-tiles (256 contraction rows each)
    et = nc.dram_tensor("et", [NKK, 128, 2, B], F8, kind="ExternalInput")
    elt = nc.dram_tensor("elt", [NKK, 128, 2, BL], F8, kind="ExternalInput")
    etwin = nc.dram_tensor("etwin", [NMT, 128, NKK, 2, WIN], F8,
                           kind="ExternalInput")
    colwin = nc.dram_tensor("colwin", [NMT, 128, WIN], BF16, kind="ExternalInput")
    meta = nc.dram_tensor("meta", [NMT, 128, 4], F32, kind="ExternalInput")
    out = nc.dram_tensor("out", [128, NMT], F32, kind="ExternalOutput")

    AF = mybir.ActivationFunctionType
    OP = mybir.AluOpType

    with tile.TileContext(nc) as tc:
        with (
            tc.tile_pool(name="const", bufs=1) as cpool,
            tc.tile_pool(name="psum", bufs=3, space=bass.MemorySpace.PSUM) as ppool,
            tc.tile_pool(name="psumw", bufs=2, space=bass.MemorySpace.PSUM) as pwpool,
            tc.tile_pool(name="chunks", bufs=3) as chpool,
            tc.tile_pool(name="winp", bufs=2) as wpool,
            tc.tile_pool(name="small", bufs=2) as smpool,
        ):
            ets = [cpool.tile([128, 2, B], F8, tag=f"ets{k}", name=f"ets{k}")
                   for k in range(NKK)]
            eltt = [cpool.tile([128, 2, BL], F8, tag=f"elt{k}", name=f"elt{k}")
                    for k in range(NKK)]
            etw_sb = [cpool.tile([128, NKK, 2, WIN], F8, tag=f"etw{m}",
                                 name=f"etw{m}") for m in range(NMT)]
            colw_sb = [cpool.tile([128, WIN], BF16, tag=f"colw{m}", name=f"colw{m}")
                       for m in range(NMT)]
            meta_sb = [cpool.tile([128, 4], F32, tag=f"meta{m}", name=f"meta{m}")
                       for m in range(NMT)]

            # Loads on the two HWDGE queues (SP + Act); gpsimd SWDGE issue
            # is ~1us/DMA and would gate the pipeline. Each ets k-tile is one
            # fully-contiguous DMA; queues alternate so transfers parallelize.
            for k in range(NKK):
                eng = nc.sync if k % 2 == 0 else nc.scalar
                eng.dma_start(eltt[k][:], elt[k])
                eng.dma_start(ets[k][:], et[k])
            for m in range(NMT):
                eng = nc.sync if m % 2 == 0 else nc.scalar
                eng.dma_start(meta_sb[m][:], meta[m])
                eng.dma_start(colw_sb[m][:], colwin[m])
                eng.dma_start(etw_sb[m][:], etwin[m])

            sexps, mews, denoms, invs = [], [], [], []
            # ---- Phase A (Exp table set): dense accums + window pipeline --
            for mt in range(NMT):
                rowlab = meta_sb[mt][:, 0:1]
                aparts = smpool.tile([128, NCH], F32, tag="aparts")

                # dense: sum_j exp(s_ij) via fused accumulate, output dead
                for c in range(NCH):
                    psum = ppool.tile([128, CH], F32, tag="psum")
                    for k in range(NKK):
                        lhsT = eltt[k][:, :, mt * 128:(mt + 1) * 128]
                        for h in range(CH // 512):
                            col0 = c * CH + h * 512
                            nc.tensor.matmul(
                                psum[:, h * 512:(h + 1) * 512],
                                lhsT,
                                ets[k][:, :, col0:col0 + 512],
                                start=(k == 0),
                                stop=(k == NKK - 1),
                                perf_mode=mybir.MatmulPerfMode.DoubleRow,
                            )
                    dead = chpool.tile([128, CH], BF16, tag="dead")
                    nc.scalar.activation(
                        dead[:], psum[:], AF.Exp,
                        scale=TINV / (SCALE * SCALE),
                        accum_out=aparts[:, c:c + 1],
                    )

                # window: recompute the <=512 same-label columns
                psw = pwpool.tile([128, WIN], F32, tag="psw")
                for k in range(NKK):
                    nc.tensor.matmul(
                        psw[:],
                        eltt[k][:, :, mt * 128:(mt + 1) * 128],
                        etw_sb[mt][:, k],
                        start=(k == 0),
                        stop=(k == NKK - 1),
                        perf_mode=mybir.MatmulPerfMode.DoubleRow,
                    )
                expw = wpool.tile([128, WIN], F32, tag="expw")
                last_a_act = nc.scalar.activation(
                    expw[:], psw[:], AF.Exp, scale=TINV / (SCALE * SCALE))
                maskw = wpool.tile([128, WIN], BF16, tag="maskw")
                nc.vector.tensor_scalar(
                    maskw[:], colw_sb[mt][:], rowlab, None, OP.is_equal)
                mew = wpool.tile([128, WIN], F32, tag=f"mew{mt}", name=f"mew{mt}",
                                 bufs=1)
                nc.vector.tensor_tensor(mew[:], expw[:], maskw[:], OP.mult)
                ssame = smpool.tile([128, 1], F32, tag="ssame")
                nc.vector.tensor_reduce(
                    ssame[:], mew[:], mybir.AxisListType.X, OP.add)

                sall = smpool.tile([128, 1], F32, tag="sall")
                nc.vector.tensor_reduce(
                    sall[:], aparts[:], mybir.AxisListType.X, OP.add)
                denom = smpool.tile([128, 1], F32, tag=f"denom{mt}",
                                    name=f"denom{mt}")
                nc.vector.tensor_sub(denom[:], sall[:], ssame[:])
                inv = smpool.tile([128, 1], F32, tag=f"inv{mt}", name=f"inv{mt}")
                nc.vector.reciprocal(inv[:], denom[:])
                mews.append(mew); denoms.append(denom); invs.append(inv)

            # ---- Phase B (Ln table set): all log work batched ----
            lnouts = wpool.tile([128, WIN], BF16, tag="lnout", bufs=1)
            rowtots = wpool.tile([128, NMT], F32, tag="rowtots", bufs=1)
            for mt in range(NMT):
                cnt = meta_sb[mt][:, 1:2]
                msum = meta_sb[mt][:, 2:3]
                eii = meta_sb[mt][:, 3:4]
                denom, inv, mew = denoms[mt], invs[mt], mews[mt]

                lnden = smpool.tile([128, 1], F32, tag=f"lnden{mt}",
                                    name=f"lnden{mt}")
                i_ld = nc.scalar.activation(lnden[:], denom[:], AF.Ln)
                tile.add_dep_helper(i_ld.ins, last_a_act.ins, sync=False,
                                    reason="keep Ln set after all Exp work")
                lndiag = smpool.tile([128, 1], F32, tag=f"lndiag{mt}",
                                     name=f"lndiag{mt}")
                i_lg = nc.scalar.activation(lndiag[:], eii, AF.Ln, bias=denom[:])
                tile.add_dep_helper(i_lg.ins, last_a_act.ins, sync=False,
                                    reason="keep Ln set after all Exp work")
                slog = smpool.tile([128, 1], F32, tag=f"slog{mt}",
                                   name=f"slog{mt}")
                i_sl = nc.scalar.activation(
                    lnouts[:], mew[:], AF.Ln,
                    scale=inv[:], bias=1.0, accum_out=slog[:],
                )
                tile.add_dep_helper(i_sl.ins, last_a_act.ins, sync=False,
                                    reason="keep Ln set after all Exp work")

                # rowtot = ((cnt*lnden + slog) - lndiag) + (sii - rds)
                ta = smpool.tile([128, 1], F32, tag=f"ta{mt}", name=f"ta{mt}")
                nc.vector.tensor_scalar(
                    ta[:], lnden[:], cnt, slog[:, 0:1], OP.mult, OP.add)
                nc.vector.tensor_scalar(
                    rowtots[:, mt:mt + 1], ta[:], lndiag[:, 0:1], msum,
                    OP.subtract, OP.add)
            nc.sync.dma_start(out[:], rowtots[:])
    nc.compile()
    return nc


def _make_in_maps(embeddings, labels):
    """Host-side prep: label-sort, transposes, windows, per-row scalars,
    per-core input dicts. Returns (in_maps, num_pos)."""
    emb0 = np.ascontiguousarray(np.asarray(embeddings, dtype=np.float32))
    lab0 = np.asarray(labels).astype(np.int64)
    assert emb0.shape == (B, D) and lab0.shape == (B,)

    perm = np.argsort(lab0, kind="stable")
    emb = emb0[perm]
    lab = lab0[perm]

    ET = np.ascontiguousarray(emb.T)                      # [D, B], sorted cols
    ET8 = (ET * SCALE).astype(NP_F8)

    def dr_pack(a):
        # [D, X] -> [NKK, 128, 2, X] with d = kk*256 + ko*128 + ki
        X = a.shape[1]
        return np.ascontiguousarray(
            a.reshape(NK // 2, 2, 128, X).transpose(0, 2, 1, 3))
    labf = lab.astype(np.float32)
    lab16 = labf.astype(ml_dtypes.bfloat16)

    ncls = int(lab.max()) + 1
    counts = np.bincount(lab, minlength=ncls)
    cum = np.concatenate([[0], np.cumsum(counts)])
    cnt = counts[lab].astype(np.float64)                  # same-label count incl. self
    num_pos = float(cnt.sum() - B)

    emb64 = emb.astype(np.float64)
    G = np.zeros((ncls, D), np.float64)
    np.add.at(G, lab, emb64)
    rds = (emb64 * G[lab]).sum(1) * TINV                  # sum_{j same} sims_ij / T
    sii = (emb64 * emb64).sum(1) * TINV                   # sims_ii / T

    meta_all = np.stack(
        [labf.astype(np.float64), cnt, sii - rds, np.exp(sii)], axis=-1
    ).astype(np.float32)                                  # [B, 4]

    in_maps = []
    for c in range(NCORES):
        sl = slice(c * BL, (c + 1) * BL)
        etwin = np.zeros((NMT, D, WIN), NP_F8)
        colwin = np.zeros((NMT, 128, WIN), ml_dtypes.bfloat16)
        colwin[:, :, :] = ml_dtypes.bfloat16(-1.0)        # never matches a label
        for m in range(NMT):
            r0 = c * BL + m * 128
            c0 = int(cum[lab[r0]])
            c1 = int(cum[lab[r0 + 127] + 1])
            w = c1 - c0
            assert w <= WIN, f"window {w} exceeds {WIN}; rebuild with larger WIN"
            etwin[m, :, :w] = ET8[:, c0:c1]
            colwin[m, :, :w] = lab16[c0:c1][None, :]
        etwin_packed = np.ascontiguousarray(
            etwin.reshape(NMT, NK // 2, 2, 128, WIN).transpose(0, 3, 1, 2, 4))
        in_maps.append({
            "et": dr_pack(ET8),
            "elt": dr_pack(np.ascontiguousarray(ET8[:, sl])),
            "etwin": etwin_packed,
            "colwin": colwin,
            "meta": np.ascontiguousarray(meta_all[sl].reshape(NMT, 128, 4)),
        })
    return in_maps, num_pos


def kernel(embeddings, labels):
    in_maps, num_pos = _make_in_maps(embeddings, labels)
    if "nc" not in _CACHE:
        _CACHE["nc"] = _build_nc()
    nc = _CACHE["nc"]
    res = run_bass_kernel_spmd(nc, in_maps, list(range(NCORES)))
    total = sum(float(r["out"].sum()) for r in res.results)
    return np.asarray(total / max(num_pos, 1.0), dtype=np.float32)



# revision 9
# speedup vs baseline: 1.0483x; 1.0483x over previous
"""Supervised contrastive loss on 8 Trainium2 NeuronCores.

Data-parallel over embedding rows (512 rows/core), label-sorted so each
128-row m-tile's same-label partners live in one <=256-column window.

Per core, per m-tile (ACT is the bottleneck engine; structure minimizes
ACT element count and instruction count):
  - window: 2 fp8 DoubleRow matmuls recompute the [128, 256] same-label
    sims; all 4 m-tiles' windows share one [128, 1024] PSUM tile drained
    by ONE Exp activate. Mask (label equality, self poisoned to -2) and
    masked row-sums run on DVE -> ssame (diagonal excluded).
  - dense: k-outer matmuls fill [128, 2048] PSUM halves (ping-pong);
    ONE Exp activate per half writes an fp16 tile; DVE row-reduces it
    (fp16 keeps the e^{s_ii}~e^10 diagonal accurate enough to subtract).
  - per-row loss, with cnt, e^{s_ii} and sum_j s_ij host-precomputed:
      denom = sall - ssame - e_ii ;  lnden = ln(denom)
      rowtot = (cnt-1)*lnden + sum_win ln(1 + mew/denom)
    (the host adds sum_rows (s_ii - sum_same s_ij) and divides by the
    exact positive count).
"""

import ml_dtypes
import numpy as np

import concourse.bass as bass
import concourse.bacc as bacc
import concourse.mybir as mybir
import concourse.tile as tile
from concourse.bass_utils import run_bass_kernel_spmd

B = 4096          # total rows
D = 512           # embedding dim
NCORES = 8
BL = B // NCORES  # rows per core
NKK = 2           # DoubleRow k-tiles (256 contraction rows each)
NMT = BL // 128   # m-tiles per core
HALF = 2048       # dense column half (4 PSUM banks)
WIN = 256         # same-label column window per m-tile (data max is 223)
TINV = 10.0       # 1 / temperature
F32 = mybir.dt.float32
F16 = mybir.dt.float16
BF16 = mybir.dt.bfloat16
F8 = mybir.dt.float8e4
NP_F8 = mybir.dt.np(F8)
SCALE = 16.0      # fp8 pre-scale; folded out via the Exp activation scale
ESC = TINV / (SCALE * SCALE)

_CACHE = {}


def _build_nc():
    nc = bacc.Bacc()
    # et packed as 4 half-column blocks, issue order (k0,h0),(k1,h0),(k0,h1),(k1,h1)
    et = nc.dram_tensor("et", [NKK * 2, 128, 2, HALF], F8, kind="ExternalInput")
    elt = nc.dram_tensor("elt", [NKK, 128, 2, BL], F8, kind="ExternalInput")
    etwin = nc.dram_tensor("etwin", [NMT, 128, NKK, 2, WIN], F8,
                           kind="ExternalInput")
    # winmeta: window labels, self position poisoned to -2, pad -1
    winmeta = nc.dram_tensor("winmeta", [128, NMT, WIN], BF16,
                             kind="ExternalInput")
    # metaf fields: 0=e^{s_ii} (device-exact), 1=row label, 2=cnt-1
    metaf = nc.dram_tensor("metaf", [128, 3, NMT], F32, kind="ExternalInput")
    out = nc.dram_tensor("out", [128, NMT], F32, kind="ExternalOutput")

    AF = mybir.ActivationFunctionType
    OP = mybir.AluOpType
    DR = mybir.MatmulPerfMode.DoubleRow
    AX = mybir.AxisListType.X

    with tile.TileContext(nc) as tc:
        with (
            tc.tile_pool(name="const", bufs=1) as cpool,
            tc.tile_pool(name="psum", bufs=2, space=bass.MemorySpace.PSUM) as ppool,
            tc.tile_pool(name="expo", bufs=2) as epool,
            tc.tile_pool(name="win", bufs=1) as wpool,
            tc.tile_pool(name="small", bufs=1) as spool,
        ):
            ets = [cpool.tile([128, 2, B], F8, tag=f"ets{k}", name=f"ets{k}")
                   for k in range(NKK)]
            eltt = [cpool.tile([128, 2, BL], F8, tag=f"elt{k}", name=f"elt{k}")
                    for k in range(NKK)]
            etw = [cpool.tile([128, NKK, 2, WIN], F8, tag=f"etw{m}",
                              name=f"etw{m}") for m in range(NMT)]
            wmeta = cpool.tile([128, NMT, WIN], BF16, tag="wmeta",
                               name="wmeta")
            mf = cpool.tile([128, 3, NMT], F32, tag="mf", name="mf")

            # DMA issue: big et blocks on the Act HWDGE ring (issued before
            # any activation work), window-phase inputs on the SP ring.
            for j in range(NKK * 2):
                k, h = j % 2, j // 2
                nc.scalar.dma_start(ets[k][:, :, h * HALF:(h + 1) * HALF],
                                    et[j])
            nc.sync.dma_start(eltt[0][:], elt[0])
            nc.sync.dma_start(eltt[1][:], elt[1])
            nc.sync.dma_start(etw[0][:], etwin[0])
            nc.sync.dma_start(wmeta[:], winmeta[:])
            nc.sync.dma_start(mf[:], metaf[:])
            for m in range(1, NMT):
                nc.sync.dma_start(etw[m][:], etwin[m])

            # ---- window sims: one PSUM tile, one Exp ----
            wp = ppool.tile([128, HALF], F32, tag="ps", name="wpsum")
            for mt in range(NMT):
                for k in range(NKK):
                    nc.tensor.matmul(
                        wp[:, mt * WIN:(mt + 1) * WIN],
                        eltt[k][:, :, mt * 128:(mt + 1) * 128],
                        etw[mt][:, k],
                        start=(k == 0), stop=(k == NKK - 1), perf_mode=DR)
            expw = wpool.tile([128, NMT, WIN], BF16, tag="expw", name="expw")
            nc.scalar.activation(expw[:], wp[:, 0:NMT * WIN], AF.Exp, scale=ESC)

            maskw = wpool.tile([128, NMT, WIN], BF16, tag="maskw", name="maskw")
            for mt in range(NMT):
                nc.vector.tensor_scalar(
                    maskw[:, mt], wmeta[:, mt],
                    mf[:, 1, mt:mt + 1], None, OP.is_equal)
            mew = wpool.tile([128, NMT, WIN], BF16, tag="mew", name="mew")
            nc.vector.tensor_tensor(mew[:], expw[:], maskw[:], OP.mult)
            ssame = spool.tile([128, NMT], F32, tag="ssame", name="ssame")
            nc.vector.tensor_reduce(ssame[:], mew[:], AX, OP.add)

            # ---- dense + per-m-tile tail ----
            rsum = spool.tile([128, NMT, 2], F32, tag="rsum", name="rsum")
            sall = spool.tile([128, NMT], F32, tag="sall", name="sall")
            denom = spool.tile([128, NMT], F32, tag="denom", name="denom")
            inv = spool.tile([128, NMT], F32, tag="inv", name="inv")
            lnden = spool.tile([128, NMT], F32, tag="lnden", name="lnden")
            slog = spool.tile([128, NMT], F32, tag="slog", name="slog")
            rowt = spool.tile([128, NMT], F32, tag="rowt", name="rowt")
            mewi = wpool.tile([128, NMT, WIN], BF16, tag="mewi", name="mewi")
            lnp = wpool.tile([128, NMT, WIN], BF16, tag="lnp", name="lnp")

            for mt in range(NMT):
                for h in range(2):
                    P = ppool.tile([128, HALF], F32, tag="ps",
                                   name=f"ps{mt}_{h}")
                    for k in range(NKK):
                        lhsT = eltt[k][:, :, mt * 128:(mt + 1) * 128]
                        for j in range(HALF // 512):
                            c = h * HALF + j * 512
                            nc.tensor.matmul(
                                P[:, j * 512:(j + 1) * 512],
                                lhsT, ets[k][:, :, c:c + 512],
                                start=(k == 0), stop=(k == NKK - 1),
                                perf_mode=DR)
                    E = epool.tile([128, HALF], F16, tag="exp",
                                   name=f"exp{mt}_{h}")
                    nc.scalar.activation(E[:], P[:], AF.Exp, scale=ESC)
                    nc.vector.tensor_reduce(rsum[:, mt, h:h + 1], E[:],
                                            AX, OP.add)
                # tail for this m-tile (overlaps later m-tiles' dense work)
                nc.vector.tensor_reduce(sall[:, mt:mt + 1], rsum[:, mt],
                                        AX, OP.add)
                nc.vector.tensor_scalar(
                    denom[:, mt:mt + 1], sall[:, mt:mt + 1],
                    ssame[:, mt:mt + 1], mf[:, 0, mt:mt + 1],
                    OP.subtract, OP.subtract)
                nc.vector.reciprocal(inv[:, mt:mt + 1], denom[:, mt:mt + 1])
                nc.scalar.activation(lnden[:, mt:mt + 1], denom[:, mt:mt + 1],
                                     AF.Ln)
                nc.vector.tensor_scalar(mewi[:, mt], mew[:, mt],
                                        inv[:, mt:mt + 1], None, OP.mult)
                nc.scalar.activation(lnp[:, mt], mewi[:, mt], AF.Ln,
                                     scale=1.0, bias=1.0)
                nc.vector.tensor_reduce(slog[:, mt:mt + 1], lnp[:, mt],
                                        AX, OP.add)
                nc.vector.tensor_scalar(
                    rowt[:, mt:mt + 1], lnden[:, mt:mt + 1],
                    mf[:, 2, mt:mt + 1], slog[:, mt:mt + 1],
                    OP.mult, OP.add)
            nc.sync.dma_start(out[:], rowt[:])
    nc.compile()
    return nc


def _make_in_maps(embeddings, labels):
    """Host prep: label-sort, fp8 transposes, windows, per-row scalars.
    Returns (in_maps, num_pos, host_extra): host_extra is added to the
    device row-total sum before dividing by num_pos."""
    emb0 = np.ascontiguousarray(np.asarray(embeddings, dtype=np.float32))
    lab0 = np.asarray(labels).astype(np.int64)
    assert emb0.shape == (B, D) and lab0.shape == (B,)

    perm = np.argsort(lab0, kind="stable")
    emb = emb0[perm]
    lab = lab0[perm]

    ET = np.ascontiguousarray(emb.T)                      # [D, B] sorted cols
    ET8 = (ET * SCALE).astype(NP_F8)

    def dr_pack(a):
        # [D, X] -> [NKK, 128, 2, X] with d = kk*256 + ko*128 + ki
        X = a.shape[1]
        return np.ascontiguousarray(
            a.reshape(NKK, 2, 128, X).transpose(0, 2, 1, 3))

    lab16 = lab.astype(np.float32).astype(ml_dtypes.bfloat16)

    ncls = int(lab.max()) + 1
    counts = np.bincount(lab, minlength=ncls)
    cum = np.concatenate([[0], np.cumsum(counts)])
    cnt = counts[lab].astype(np.float64)                  # incl. self
    num_pos = float(cnt.sum() - B)

    emb64 = emb.astype(np.float64)
    G = np.zeros((ncls, D), np.float64)
    np.add.at(G, lab, emb64)
    rds = (emb64 * G[lab]).sum(1) * TINV                  # sum_same s_ij (incl self)
    sii = (emb64 * emb64).sum(1) * TINV
    host_extra = float((sii - rds).sum())                 # sum_rows (s_ii - sum_{j!=i} s_ij)

    # device-exact e^{s_ii}: replicate the fp8 matmul's diagonal
    et64 = ET8.astype(np.float64)
    sii_dev = (et64 * et64).sum(0) * ESC                  # [B]
    eii_dev = np.exp(sii_dev).astype(np.float32)

    dr = dr_pack(ET8)                                     # [NKK, 128, 2, B]
    et_j = np.ascontiguousarray(np.stack([
        dr[0][:, :, 0:HALF], dr[1][:, :, 0:HALF],
        dr[0][:, :, HALF:], dr[1][:, :, HALF:]]))         # [4, 128, 2, HALF]

    in_maps = []
    for c in range(NCORES):
        sl = slice(c * BL, (c + 1) * BL)
        etwin = np.zeros((NMT, D, WIN), NP_F8)
        winmeta = np.zeros((128, NMT, WIN), ml_dtypes.bfloat16)
        winmeta[:] = ml_dtypes.bfloat16(-1.0)             # pad: matches no label
        metaf = np.zeros((128, 3, NMT), np.float32)
        for m in range(NMT):
            r0 = c * BL + m * 128
            c0 = int(cum[lab[r0]])
            c1 = int(cum[lab[r0 + 127] + 1])
            w = c1 - c0
            assert w <= WIN, f"window {w} exceeds {WIN}; rebuild with larger WIN"
            etwin[m, :, :w] = ET8[:, c0:c1]
            winmeta[:, m, :w] = lab16[c0:c1][None, :]
            for p in range(128):
                winmeta[p, m, r0 + p - c0] = ml_dtypes.bfloat16(-2.0)  # self
            metaf[:, 0, m] = eii_dev[r0:r0 + 128]
            metaf[:, 1, m] = lab[r0:r0 + 128].astype(np.float32)
            metaf[:, 2, m] = (cnt[r0:r0 + 128] - 1.0).astype(np.float32)
        etwin_packed = np.ascontiguousarray(
            etwin.reshape(NMT, NKK, 2, 128, WIN).transpose(0, 3, 1, 2, 4))
        in_maps.append({
            "et": et_j,
            "elt": dr_pack(np.ascontiguousarray(ET8[:, sl])),
            "etwin": etwin_packed,
            "winmeta": winmeta,
            "metaf": metaf,
        })
    return in_maps, num_pos, host_extra


def kernel(embeddings, labels):
    in_maps, num_pos, host_extra = _make_in_maps(embeddings, labels)
    if "nc" not in _CACHE:
        _CACHE["nc"] = _build_nc()
    nc = _CACHE["nc"]
    res = run_bass_kernel_spmd(nc, in_maps, list(range(NCORES)))
    total = sum(float(r["out"].sum()) for r in res.results) + host_extra
    return np.asarray(total / max(num_pos, 1.0), dtype=np.float32)


# revision 15
# speedup vs baseline: 1.1907x; 1.1358x over previous
"""Supervised contrastive loss on 8 Trainium2 NeuronCores.

Data-parallel over embedding rows (512 rows/core), label-sorted so each
128-row m-tile's same-label partners live in one <=256-column window.

Per core, per m-tile (ACT is the bottleneck engine; structure minimizes
ACT element count and instruction count):
  - window: 2 fp8 DoubleRow matmuls recompute the [128, 256] same-label
    sims; all 4 m-tiles' windows share one [128, 1024] PSUM tile drained
    by ONE Exp activate. Mask (label equality, self poisoned to -2) and
    masked row-sums run on DVE -> ssame (diagonal excluded).
  - dense: k-outer matmuls fill [128, 2048] PSUM halves (ping-pong);
    ONE Exp activate per half writes an fp16 tile; DVE row-reduces it
    (fp16 keeps the e^{s_ii}~e^10 diagonal accurate enough to subtract).
  - per-row loss, with cnt, e^{s_ii} and sum_j s_ij host-precomputed:
      denom = sall - ssame - e_ii ;  lnden = ln(denom)
      rowtot = (cnt-1)*lnden + sum_win ln(1 + mew/denom)
    (the host adds sum_rows (s_ii - sum_same s_ij) and divides by the
    exact positive count).
"""

import ml_dtypes
import numpy as np

import concourse.bass as bass
import concourse.bacc as bacc
import concourse.mybir as mybir
import concourse.tile as tile
from concourse.bass_utils import run_bass_kernel_spmd
from concourse.hw_specs import get_activation_tables

B = 4096          # total rows
D = 512           # embedding dim
NCORES = 8
BL = B // NCORES  # rows per core
NKK = 2           # DoubleRow k-tiles (256 contraction rows each)
NMT = BL // 128   # m-tiles per core
HALF = 2048       # dense column half (4 PSUM banks)
WIN = 256         # same-label column window per m-tile (data max is 223)
MMW = 512        # matmul output width (columns per matmul instruction)
TINV = 10.0       # 1 / temperature
F32 = mybir.dt.float32
F16 = mybir.dt.float16
BF16 = mybir.dt.bfloat16
F8 = mybir.dt.float8e4
NP_F8 = mybir.dt.np(F8)
SCALE = 16.0      # fp8 pre-scale; folded out via the Exp activation scale
ESC = TINV / (SCALE * SCALE)

_CACHE = {}


def _build_nc():
    nc = bacc.Bacc()
    # et packed as 4 half-column blocks, issue order (k0,h0),(k1,h0),(k0,h1),(k1,h1)
    et = nc.dram_tensor("et", [NKK * 2, 128, 2, HALF], F8, kind="ExternalInput")
    elt = nc.dram_tensor("elt", [NKK, 128, 2, BL], F8, kind="ExternalInput")
    etwin = nc.dram_tensor("etwin", [NMT, 128, NKK, 2, WIN], F8,
                           kind="ExternalInput")
    # winmeta: window labels, self position poisoned to -2, pad -1
    winmeta = nc.dram_tensor("winmeta", [128, NMT, WIN], BF16,
                             kind="ExternalInput")
    # metaf fields: 0=e^{s_ii} (device-exact), 1=row label, 2=cnt-1
    metaf = nc.dram_tensor("metaf", [128, 3, NMT], F32, kind="ExternalInput")
    out = nc.dram_tensor("out", [128, NMT], F32, kind="ExternalOutput")

    AF = mybir.ActivationFunctionType
    OP = mybir.AluOpType
    DR = mybir.MatmulPerfMode.DoubleRow
    AX = mybir.AxisListType.X

    with tile.TileContext(nc) as tc:
        with (
            tc.tile_pool(name="const", bufs=1) as cpool,
            tc.tile_pool(name="psum", bufs=2, space=bass.MemorySpace.PSUM) as ppool,
            tc.tile_pool(name="expo", bufs=2) as epool,
            tc.tile_pool(name="win", bufs=1) as wpool,
            tc.tile_pool(name="small", bufs=1) as spool,
        ):
            ets = [cpool.tile([128, 2, B], F8, tag=f"ets{k}", name=f"ets{k}")
                   for k in range(NKK)]
            eltt = [cpool.tile([128, 2, BL], F8, tag=f"elt{k}", name=f"elt{k}")
                    for k in range(NKK)]
            etw = [cpool.tile([128, NKK, 2, WIN], F8, tag=f"etw{m}",
                              name=f"etw{m}") for m in range(NMT)]
            wmeta = cpool.tile([128, NMT, WIN], BF16, tag="wmeta",
                               name="wmeta")
            mf = cpool.tile([128, 3, NMT], F32, tag="mf", name="mf")

            # DMA issue: big et blocks on the Act HWDGE ring (issued before
            # any activation work), window-phase inputs on the SP ring.
            for j in range(NKK * 2):
                k, h = j % 2, j // 2
                nc.scalar.dma_start(ets[k][:, :, h * HALF:(h + 1) * HALF],
                                    et[j])
            # one table load serves both Exp and Ln (avoids set thrash)
            tabs = list(get_activation_tables(nc.m.arch).keys())
            nc.scalar.add_instruction(mybir.InstLoadActFuncSet(
                name=nc.get_next_instruction_name(), ins=[], outs=[],
                act_func_set_id=tabs.index("natural_log_exp_and_others")))
            nc.sync.dma_start(eltt[0][:], elt[0])
            nc.sync.dma_start(eltt[1][:], elt[1])
            nc.sync.dma_start(etw[0][:], etwin[0])
            nc.sync.dma_start(wmeta[:], winmeta[:])
            nc.sync.dma_start(mf[:], metaf[:])
            for m in range(1, NMT):
                nc.sync.dma_start(etw[m][:], etwin[m])

            # ---- window sims: one PSUM tile, one Exp ----
            wp = ppool.tile([128, HALF], F32, tag="ps", name="wpsum")
            for mt in range(NMT):
                for k in range(NKK):
                    nc.tensor.matmul(
                        wp[:, mt * WIN:(mt + 1) * WIN],
                        eltt[k][:, :, mt * 128:(mt + 1) * 128],
                        etw[mt][:, k],
                        start=(k == 0), stop=(k == NKK - 1), perf_mode=DR)
            expw = wpool.tile([128, NMT, WIN], BF16, tag="expw", name="expw")
            nc.scalar.activation(expw[:], wp[:, 0:NMT * WIN], AF.Exp, scale=ESC)

            maskw = wpool.tile([128, NMT, WIN], BF16, tag="maskw", name="maskw")
            for mt in range(NMT):
                nc.vector.tensor_scalar(
                    maskw[:, mt], wmeta[:, mt],
                    mf[:, 1, mt:mt + 1], None, OP.is_equal)
            mew = wpool.tile([128, NMT, WIN], BF16, tag="mew", name="mew")
            nc.vector.tensor_tensor(mew[:], expw[:], maskw[:], OP.mult)
            ssame = spool.tile([128, NMT], F32, tag="ssame", name="ssame")
            nc.vector.tensor_reduce(ssame[:], mew[:], AX, OP.add)

            # ---- dense + per-m-tile tail ----
            rsum = spool.tile([128, NMT, 2], F32, tag="rsum", name="rsum")
            sall = spool.tile([128, NMT], F32, tag="sall", name="sall")
            denom = spool.tile([128, NMT], F32, tag="denom", name="denom")
            inv = spool.tile([128, NMT], F32, tag="inv", name="inv")
            lnden = spool.tile([128, NMT], F32, tag="lnden", name="lnden")
            slog = spool.tile([128, NMT], F32, tag="slog", name="slog")
            rowt = spool.tile([128, NMT], F32, tag="rowt", name="rowt")
            mewi = wpool.tile([128, NMT, WIN], BF16, tag="mewi", name="mewi")
            lnp = wpool.tile([128, NMT, WIN], BF16, tag="lnp", name="lnp")

            for mt in range(NMT):
                for h in range(2):
                    P = ppool.tile([128, HALF], F32, tag="ps",
                                   name=f"ps{mt}_{h}")
                    for k in range(NKK):
                        lhsT = eltt[k][:, :, mt * 128:(mt + 1) * 128]
                        for j in range(HALF // MMW):
                            c = h * HALF + j * MMW
                            nc.tensor.matmul(
                                P[:, j * MMW:(j + 1) * MMW],
                                lhsT, ets[k][:, :, c:c + MMW],
                                start=(k == 0), stop=(k == NKK - 1),
                                perf_mode=DR)
                    E = epool.tile([128, HALF], BF16, tag="exp",
                                   name=f"exp{mt}_{h}")
                    nc.scalar.activation(E[:], P[:], AF.Exp, scale=ESC,
                                         accum_out=rsum[:, mt, h:h + 1])
                # tail for this m-tile (overlaps later m-tiles' dense work)
                nc.vector.tensor_reduce(sall[:, mt:mt + 1], rsum[:, mt],
                                        AX, OP.add)
                nc.vector.tensor_scalar(
                    denom[:, mt:mt + 1], sall[:, mt:mt + 1],
                    ssame[:, mt:mt + 1], mf[:, 0, mt:mt + 1],
                    OP.subtract, OP.subtract)
                nc.vector.reciprocal(inv[:, mt:mt + 1], denom[:, mt:mt + 1])
                nc.scalar.activation(lnden[:, mt:mt + 1], denom[:, mt:mt + 1],
                                     AF.Ln)
                nc.vector.tensor_scalar(mewi[:, mt], mew[:, mt],
                                        inv[:, mt:mt + 1], None, OP.mult)
                nc.scalar.activation(lnp[:, mt], mewi[:, mt], AF.Ln,
                                     scale=1.0, bias=1.0)
                nc.vector.tensor_reduce(slog[:, mt:mt + 1], lnp[:, mt],
                                        AX, OP.add)
                nc.vector.tensor_scalar(
                    rowt[:, mt:mt + 1], lnden[:, mt:mt + 1],
                    mf[:, 2, mt:mt + 1], slog[:, mt:mt + 1],
                    OP.mult, OP.add)
            nc.sync.dma_start(out[:], rowt[:])
    nc.compile()
    return nc


def _make_in_maps(embeddings, labels):
    """Host prep: label-sort, fp8 transposes, windows, per-row scalars.
    Returns (in_maps, num_pos, host_extra): host_extra is added to the
    device row-total sum before dividing by num_pos."""
    emb0 = np.ascontiguousarray(np.asarray(embeddings, dtype=np.float32))
    lab0 = np.asarray(labels).astype(np.int64)
    assert emb0.shape == (B, D) and lab0.shape == (B,)

    perm = np.argsort(lab0, kind="stable")
    emb = emb0[perm]
    lab = lab0[perm]

    ET = np.ascontiguousarray(emb.T)                      # [D, B] sorted cols
    ET8 = (ET * SCALE).astype(NP_F8)

    def dr_pack(a):
        # [D, X] -> [NKK, 128, 2, X] with d = kk*256 + ko*128 + ki
        X = a.shape[1]
        return np.ascontiguousarray(
            a.reshape(NKK, 2, 128, X).transpose(0, 2, 1, 3))

    lab16 = lab.astype(np.float32).astype(ml_dtypes.bfloat16)

    ncls = int(lab.max()) + 1
    counts = np.bincount(lab, minlength=ncls)
    cum = np.concatenate([[0], np.cumsum(counts)])
    cnt = counts[lab].astype(np.float64)                  # incl. self
    num_pos = float(cnt.sum() - B)

    emb64 = emb.astype(np.float64)
    G = np.zeros((ncls, D), np.float64)
    np.add.at(G, lab, emb64)
    rds = (emb64 * G[lab]).sum(1) * TINV                  # sum_same s_ij (incl self)
    sii = (emb64 * emb64).sum(1) * TINV
    host_extra = float((sii - rds).sum())                 # sum_rows (s_ii - sum_{j!=i} s_ij)

    # device-exact e^{s_ii}: replicate the fp8 matmul's diagonal
    et64 = ET8.astype(np.float64)
    sii_dev = (et64 * et64).sum(0) * ESC                  # [B]
    eii_dev = np.exp(sii_dev).astype(np.float32)

    dr = dr_pack(ET8)                                     # [NKK, 128, 2, B]
    et_j = np.ascontiguousarray(np.stack([
        dr[0][:, :, 0:HALF], dr[1][:, :, 0:HALF],
        dr[0][:, :, HALF:], dr[1][:, :, HALF:]]))         # [4, 128, 2, HALF]

    in_maps = []
    for c in range(NCORES):
        sl = slice(c * BL, (c + 1) * BL)
        etwin = np.zeros((NMT, D, WIN), NP_F8)
        winmeta = np.zeros((128, NMT, WIN), ml_dtypes.bfloat16)
        winmeta[:] = ml_dtypes.bfloat16(-1.0)             # pad: matches no label
        metaf = np.zeros((128, 3, NMT), np.float32)
        for m in range(NMT):
            r0 = c * BL + m * 128
            c0 = int(cum[lab[r0]])
            c1 = int(cum[lab[r0 + 127] + 1])
            w = c1 - c0
            assert w <= WIN, f"window {w} exceeds {WIN}; rebuild with larger WIN"
            etwin[m, :, :w] = ET8[:, c0:c1]
            winmeta[:, m, :w] = lab16[c0:c1][None, :]
            for p in range(128):
                winmeta[p, m, r0 + p - c0] = ml_dtypes.bfloat16(-2.0)  # self
            metaf[:, 0, m] = eii_dev[r0:r0 + 128]
            metaf[:, 1, m] = lab[r0:r0 + 128].astype(np.float32)
            metaf[:, 2, m] = (cnt[r0:r0 + 128] - 1.0).astype(np.float32)
        etwin_packed = np.ascontiguousarray(
            etwin.reshape(NMT, NKK, 2, 128, WIN).transpose(0, 3, 1, 2, 4))
        in_maps.append({
            "et": et_j,
            "elt": dr_pack(np.ascontiguousarray(ET8[:, sl])),
            "etwin": etwin_packed,
            "winmeta": winmeta,
            "metaf": metaf,
        })
    return in_maps, num_pos, host_extra


def kernel(embeddings, labels):
    in_maps, num_pos, host_extra = _make_in_maps(embeddings, labels)
    if "nc" not in _CACHE:
        _CACHE["nc"] = _build_nc()
    nc = _CACHE["nc"]
    res = run_bass_kernel_spmd(nc, in_maps, list(range(NCORES)))
    total = sum(float(r["out"].sum()) for r in res.results) + host_extra
    return np.asarray(total / max(num_pos, 1.0), dtype=np.float32)


# revision 20
# speedup vs baseline: 1.3130x; 1.1027x over previous
"""Supervised contrastive loss on 8 Trainium2 NeuronCores.

Data-parallel over embedding rows (512 rows/core), label-sorted so each
128-row m-tile's same-label partners live in one <=256-column window.

Per core, per m-tile (ACT is the bottleneck engine; structure minimizes
ACT element count and instruction count):
  - window: 2 fp8 DoubleRow matmuls recompute the [128, 256] same-label
    sims; all 4 m-tiles' windows share one [128, 1024] PSUM tile drained
    by ONE Exp activate. Mask (label equality, self poisoned to -2) and
    masked row-sums run on DVE -> ssame (diagonal excluded).
  - dense: k-outer matmuls fill [128, 2048] PSUM halves (ping-pong);
    ONE Exp activate per half writes an fp16 tile; DVE row-reduces it
    (fp16 keeps the e^{s_ii}~e^10 diagonal accurate enough to subtract).
  - per-row loss, with cnt, e^{s_ii} and sum_j s_ij host-precomputed:
      denom = sall - ssame - e_ii ;  lnden = ln(denom)
      rowtot = (cnt-1)*lnden + sum_win ln(1 + mew/denom)
    (the host adds sum_rows (s_ii - sum_same s_ij) and divides by the
    exact positive count).
"""

import ml_dtypes
import numpy as np

import concourse.bass as bass
import concourse.bacc as bacc
import concourse.mybir as mybir
import concourse.tile as tile
from concourse.bass_utils import run_bass_kernel_spmd
from concourse.hw_specs import get_activation_tables

B = 4096          # total rows
D = 512           # embedding dim
NCORES = 8
BL = B // NCORES  # rows per core
NKK = 2           # DoubleRow k-tiles (256 contraction rows each)
NMT = BL // 128   # m-tiles per core
HALF = 2048       # dense column half (4 PSUM banks)
WIN = 256         # same-label column window per m-tile (data max is 223)
MMW = 512        # matmul output width (columns per matmul instruction)
TINV = 10.0       # 1 / temperature
F32 = mybir.dt.float32
F16 = mybir.dt.float16
BF16 = mybir.dt.bfloat16
F8 = mybir.dt.float8e4
NP_F8 = mybir.dt.np(F8)
SCALE = 16.0      # fp8 pre-scale; folded out via the Exp activation scale
ESC = TINV / (SCALE * SCALE)

_CACHE = {}


def _build_nc():
    nc = bacc.Bacc()
    # et packed as 4 half-column blocks, issue order (k0,h0),(k1,h0),(k0,h1),(k1,h1)
    et = nc.dram_tensor("et", [NKK * 2, 128, 2, HALF], F8, kind="ExternalInput")
    # pack8: one full-line DMA for all small fp8 inputs.
    # j=0,1: own-row lhsT k-tiles; j=2..5: window rhs for m-tile j-2
    pack8 = nc.dram_tensor("pack8", [128, 2 + NMT, 2, 2, WIN], F8,
                           kind="ExternalInput")
    # pack16: window labels (self poisoned -2, pad -1) + per-row scalars
    # [..., WIN]=row label, [WIN+1]=cnt-1, [WIN+2]=eii_hi, [WIN+3]=eii_lo
    pack16 = nc.dram_tensor("pack16", [128, NMT, WIN + 4], BF16,
                            kind="ExternalInput")
    out = nc.dram_tensor("out", [128, NMT], F32, kind="ExternalOutput")

    AF = mybir.ActivationFunctionType
    OP = mybir.AluOpType
    DR = mybir.MatmulPerfMode.DoubleRow
    AX = mybir.AxisListType.X

    with tile.TileContext(nc) as tc:
        with (
            tc.tile_pool(name="const", bufs=1) as cpool,
            tc.tile_pool(name="psum", bufs=2, space=bass.MemorySpace.PSUM) as ppool,
            tc.tile_pool(name="expo", bufs=2) as epool,
            tc.tile_pool(name="win", bufs=1) as wpool,
            tc.tile_pool(name="small", bufs=1) as spool,
        ):
            # ets[k] laid out [128, half, kk, HALF] so each half-block DMA
            # lands in one contiguous 4 KiB line per partition
            ets = [cpool.tile([128, 2, 2, HALF], F8, tag=f"ets{k}",
                              name=f"ets{k}") for k in range(NKK)]
            pk8 = cpool.tile([128, 2 + NMT, 2, 2, WIN], F8, tag="pk8",
                             name="pk8")
            pk16 = cpool.tile([128, NMT, WIN + 4], BF16, tag="pk16",
                              name="pk16")

            def lhsT(k, mt):
                # own-row weights for m-tile mt: [128, 2, 128]
                return pk8[:, k, :, mt // 2,
                           (mt % 2) * 128:(mt % 2) * 128 + 128]

            def rhs(k, h, c, w):
                # dense columns [h*HALF+c, +w): [128, 2, w]
                return ets[k][:, h, :, c:c + w]

            # DMA issue: big et blocks on the Act HWDGE ring (issued before
            # any activation work), packed small inputs on the SP ring.
            for j in range(2):
                k, h = j % 2, j // 2
                nc.scalar.dma_start(ets[k][:, h], et[j])
            # one table load serves both Exp and Ln (avoids set thrash)
            tabs = list(get_activation_tables(nc.m.arch).keys())
            nc.scalar.add_instruction(mybir.InstLoadActFuncSet(
                name=nc.get_next_instruction_name(), ins=[], outs=[],
                act_func_set_id=tabs.index("natural_log_exp_and_others")))
            for j in range(2, NKK * 2):
                k, h = j % 2, j // 2
                nc.scalar.dma_start(ets[k][:, h], et[j])
            nc.sync.dma_start(pk8[:], pack8[:])
            nc.sync.dma_start(pk16[:], pack16[:])

            # ---- window sims: one PSUM tile, one Exp ----
            wp = ppool.tile([128, HALF], F32, tag="ps", name="wpsum")
            for mt in range(NMT):
                for k in range(NKK):
                    nc.tensor.matmul(
                        wp[:, mt * WIN:(mt + 1) * WIN],
                        lhsT(k, mt), pk8[:, 2 + mt, k],
                        start=(k == 0), stop=(k == NKK - 1), perf_mode=DR)
            expw = wpool.tile([128, NMT, WIN], BF16, tag="expw", name="expw")
            nc.scalar.activation(expw[:], wp[:, 0:NMT * WIN], AF.Exp, scale=ESC)

            # per-row scalars to f32: [...,0]=row label, [...,1]=cnt-1
            scal = spool.tile([128, NMT, 2], F32, tag="scal", name="scal")
            nc.vector.tensor_scalar(scal[:], pk16[:, :, WIN:WIN + 2], 0.0,
                                    None, OP.add)
            # e^{s_ii} from its two bf16 halves
            eii = spool.tile([128, NMT], F32, tag="eii", name="eii")
            nc.vector.tensor_reduce(eii[:], pk16[:, :, WIN + 2:WIN + 4],
                                    AX, OP.add)

            maskw = wpool.tile([128, NMT, WIN], BF16, tag="maskw", name="maskw")
            for mt in range(NMT):
                nc.vector.tensor_scalar(
                    maskw[:, mt], pk16[:, mt, 0:WIN],
                    scal[:, mt, 0:1], None, OP.is_equal)
            mew = wpool.tile([128, NMT, WIN], BF16, tag="mew", name="mew")
            nc.vector.tensor_tensor(mew[:], expw[:], maskw[:], OP.mult)
            ssame = spool.tile([128, NMT], F32, tag="ssame", name="ssame")
            nc.vector.tensor_reduce(ssame[:], mew[:], AX, OP.add)

            # ---- dense + per-m-tile tail ----
            rsum = spool.tile([128, NMT, 2], F32, tag="rsum", name="rsum")
            sall = spool.tile([128, NMT], F32, tag="sall", name="sall")
            denom = spool.tile([128, NMT], F32, tag="denom", name="denom")
            inv = spool.tile([128, NMT], F32, tag="inv", name="inv")
            lnden = spool.tile([128, NMT], F32, tag="lnden", name="lnden")
            slog = spool.tile([128, NMT], F32, tag="slog", name="slog")
            rowt = spool.tile([128, NMT], F32, tag="rowt", name="rowt")
            mewi = wpool.tile([128, NMT, WIN], BF16, tag="mewi", name="mewi")
            lnp = wpool.tile([128, NMT, WIN], BF16, tag="lnp", name="lnp")

            for mt in range(NMT):
                for h in range(2):
                    P = ppool.tile([128, HALF], F32, tag="ps",
                                   name=f"ps{mt}_{h}")
                    for k in range(NKK):
                        w = lhsT(k, mt)
                        for j in range(HALF // MMW):
                            nc.tensor.matmul(
                                P[:, j * MMW:(j + 1) * MMW],
                                w, rhs(k, h, j * MMW, MMW),
                                start=(k == 0), stop=(k == NKK - 1),
                                perf_mode=DR)
                    E = epool.tile([128, HALF], BF16, tag="exp",
                                   name=f"exp{mt}_{h}")
                    nc.scalar.activation(E[:], P[:], AF.Exp, scale=ESC,
                                         accum_out=rsum[:, mt, h:h + 1])
                # tail for this m-tile (overlaps later m-tiles' dense work)
                nc.vector.tensor_reduce(sall[:, mt:mt + 1], rsum[:, mt],
                                        AX, OP.add)
                nc.vector.tensor_scalar(
                    denom[:, mt:mt + 1], sall[:, mt:mt + 1],
                    ssame[:, mt:mt + 1], eii[:, mt:mt + 1],
                    OP.subtract, OP.subtract)
                nc.vector.reciprocal(inv[:, mt:mt + 1], denom[:, mt:mt + 1])
                nc.scalar.activation(lnden[:, mt:mt + 1], denom[:, mt:mt + 1],
                                     AF.Ln)
                nc.vector.tensor_scalar(mewi[:, mt], mew[:, mt],
                                        inv[:, mt:mt + 1], None, OP.mult)
                nc.scalar.activation(lnp[:, mt], mewi[:, mt], AF.Ln,
                                     scale=1.0, bias=1.0)
                nc.vector.tensor_reduce(slog[:, mt:mt + 1], lnp[:, mt],
                                        AX, OP.add)
                nc.vector.tensor_scalar(
                    rowt[:, mt:mt + 1], lnden[:, mt:mt + 1],
                    scal[:, mt, 1:2], slog[:, mt:mt + 1],
                    OP.mult, OP.add)
            nc.scalar.dma_start(out[:], rowt[:])
    nc.compile()
    return nc


def _make_in_maps(embeddings, labels):
    """Host prep: label-sort, fp8 transposes, windows, per-row scalars.
    Returns (in_maps, num_pos, host_extra): host_extra is added to the
    device row-total sum before dividing by num_pos."""
    emb0 = np.ascontiguousarray(np.asarray(embeddings, dtype=np.float32))
    lab0 = np.asarray(labels).astype(np.int64)
    assert emb0.shape == (B, D) and lab0.shape == (B,)

    perm = np.argsort(lab0, kind="stable")
    emb = emb0[perm]
    lab = lab0[perm]

    ET = np.ascontiguousarray(emb.T)                      # [D, B] sorted cols
    ET8 = (ET * SCALE).astype(NP_F8)

    def dr_pack(a):
        # [D, X] -> [NKK, 128, 2, X] with d = kk*256 + ko*128 + ki
        X = a.shape[1]
        return np.ascontiguousarray(
            a.reshape(NKK, 2, 128, X).transpose(0, 2, 1, 3))

    lab16 = lab.astype(np.float32).astype(ml_dtypes.bfloat16)

    ncls = int(lab.max()) + 1
    counts = np.bincount(lab, minlength=ncls)
    cum = np.concatenate([[0], np.cumsum(counts)])
    cnt = counts[lab].astype(np.float64)                  # incl. self
    num_pos = float(cnt.sum() - B)

    emb64 = emb.astype(np.float64)
    G = np.zeros((ncls, D), np.float64)
    np.add.at(G, lab, emb64)
    rds = (emb64 * G[lab]).sum(1) * TINV                  # sum_same s_ij (incl self)
    sii = (emb64 * emb64).sum(1) * TINV
    host_extra = float((sii - rds).sum())                 # sum_rows (s_ii - sum_{j!=i} s_ij)

    # device-exact e^{s_ii}: replicate the fp8 matmul's diagonal
    et64 = ET8.astype(np.float64)
    sii_dev = (et64 * et64).sum(0) * ESC                  # [B]
    eii_dev = np.exp(sii_dev).astype(np.float32)

    dr = dr_pack(ET8)                                     # [NKK, 128, 2, B]
    et_j = np.ascontiguousarray(np.stack([
        dr[0][:, :, 0:HALF], dr[1][:, :, 0:HALF],
        dr[0][:, :, HALF:], dr[1][:, :, HALF:]]))         # [4, 128, 2, HALF]

    bf = ml_dtypes.bfloat16
    in_maps = []
    for c in range(NCORES):
        sl = slice(c * BL, (c + 1) * BL)
        etwin = np.zeros((NMT, D, WIN), NP_F8)
        pack16 = np.zeros((128, NMT, WIN + 4), bf)
        pack16[:, :, :WIN] = bf(-1.0)                     # pad: matches no label
        for m in range(NMT):
            r0 = c * BL + m * 128
            c0 = int(cum[lab[r0]])
            c1 = int(cum[lab[r0 + 127] + 1])
            w = c1 - c0
            assert w <= WIN, f"window {w} exceeds {WIN}; rebuild with larger WIN"
            etwin[m, :, :w] = ET8[:, c0:c1]
            pack16[:, m, :w] = lab16[c0:c1][None, :]
            for p in range(128):
                pack16[p, m, r0 + p - c0] = bf(-2.0)      # poison self
            pack16[:, m, WIN] = lab16[r0:r0 + 128]
            pack16[:, m, WIN + 1] = (cnt[r0:r0 + 128] - 1.0).astype(bf)
            ehi = eii_dev[r0:r0 + 128].astype(bf)
            pack16[:, m, WIN + 2] = ehi
            pack16[:, m, WIN + 3] = (
                eii_dev[r0:r0 + 128] - ehi.astype(np.float32)).astype(bf)
        # pack8: [128, 2+NMT, 2, 2, WIN]; j=0,1 own-row lhsT; j=2+m window rhs
        pack8 = np.zeros((128, 2 + NMT, 2, 2, WIN), NP_F8)
        eltp = dr_pack(np.ascontiguousarray(ET8[:, sl]))  # [NKK, 128, 2, BL]
        for k in range(NKK):
            pack8[:, k] = eltp[k].reshape(128, 2, 2, WIN)
        etwin_packed = etwin.reshape(NMT, NKK, 2, 128, WIN).transpose(
            0, 3, 1, 2, 4)                                # [NMT, 128, 2, 2, WIN]
        for m in range(NMT):
            pack8[:, 2 + m] = etwin_packed[m]
        in_maps.append({
            "et": et_j,
            "pack8": pack8,
            "pack16": pack16,
        })
    return in_maps, num_pos, host_extra


def kernel(embeddings, labels):
    in_maps, num_pos, host_extra = _make_in_maps(embeddings, labels)
    if "nc" not in _CACHE:
        _CACHE["nc"] = _build_nc()
    nc = _CACHE["nc"]
    res = run_bass_kernel_spmd(nc, in_maps, list(range(NCORES)))
    total = sum(float(r["out"].sum()) for r in res.results) + host_extra
    return np.asarray(total / max(num_pos, 1.0), dtype=np.float32)
